# revision 18
# baseline (speedup 1.0000x reference)
"""Trainium2 Bass kernel: full (non-causal) softmax attention.

Input:  query/key/value [1, 4096, 16, 128] f32 (B, S, H, D).
Output: [1, 4096, 16, 128] f32 = softmax(Q K^T / sqrt(D)) V per head.

Sharding: 16 heads over 8 cores -> 2 heads per core, no collectives.
Host pre-transposes Q,K per head to chunked [D, S] fp16; the device
returns the UN-normalized attention output [D, S] f32 plus the softmax
denominator row [S] f32; the host does the final divide.

Device pipeline, per head, per query-chunk QC (1024 queries), kt in 32
key-chunks (128 keys):
  ST[kt] = scores^T: psum f32 [128k, QC]   (two N=512 fp16 matmuls)
  PT[kt] = exp(ST/sqrt(128)) -> sbuf fp16:
     - most tiles on ACT (the 1.0us/tile throughput floor)
     - DVE_KTS tiles on DVE instead: two tensor_scalar exp2 bit-trick
       terms (i16 = st*a + b_i, bitcast fp16) summed with weights
       2^d1+2^d2 = 1/(1+mu) -- a mean-corrected two-phase Schraudolph
       with |rel err| <= 1.5% on those chunks only (~4e-3 end-to-end)
  OUT += V_kt^T @ PT[kt]   (fp16 matmuls, fp32 psum, issued PV_LAG late)
  den: PT accumulated in three fp16 chains (2x DVE, 1x GPSIMD), merged,
  then GPSIMD partition_all_reduce -> den row (no PSUM banks needed,
  freeing st to triple-buffer).
Epilogues are deferred into the next chunk's iterations; the last chunk
folds den via ones-matmuls into spare psum and drains on ACT (idle).
"""

import os
import sys
from contextlib import ExitStack

import numpy as np

sys.path.insert(0, "/opt/trn_rl_repo")

import concourse.bacc as bacc
import concourse.bass as bass
import concourse.bass_isa as bass_isa
import concourse.tile as tile
from concourse import mybir
from concourse.bass_utils import run_bass_kernel_spmd

N_CORES = 8
S = 4096
H = 16
D = 128
HEADS_PER_CORE = H // N_CORES  # 2
KT_CHUNK = 128                  # keys per score tile (psum partition dim)
QC = 1024                       # queries per super-chunk (ACT tile free dim)
NMM = 512                       # moving free dim per matmul (psum bank, f32)
SCALE = float(D) ** -0.5
LOG2E = 1.4426950408889634
PV_LAG = 6                      # PV matmuls trail QK by this many kt steps
CHAIN_LAG = 4                   # den-chain adds trail pt production, so the
                                # DVE-exp psum reads sit early in DVE's FIFO

# two-term Schraudolph constants (fp16 bit layout: bias 15, 10-bit mantissa)
# weights 2^D1 + 2^D2 = 1/(1+mu) cancel the mean of the mantissa-linear
# error; the half-period phase offset cancels its fundamental oscillation.
_SCH_MU = 0.04068
_SCH_D1 = -np.log2(1.0 + np.sqrt(2.0)) - np.log2(1.0 + _SCH_MU)
_SCH_D2 = _SCH_D1 + 0.5
SCH_A = float(SCALE * LOG2E * 1024.0)
SCH_B1 = float((15.0 + _SCH_D1) * 1024.0)
SCH_B2 = float((15.0 + _SCH_D2) * 1024.0)

DVE_KTS = (10, 21)              # kt tiles whose exp runs on DVE
GPS_KTS = ()                    # kt tiles whose den-chain add is GPSIMD
                                # (GPS adds contend with DVE's SBUF port)

F32 = mybir.dt.float32
F16 = mybir.dt.float16
I16 = mybir.dt.int16


def build_program(s=S, heads=HEADS_PER_CORE):
    nc = bacc.Bacc("TRN2", target_bir_lowering=False, debug=False,
                   num_devices=N_CORES)

    n_kt = s // KT_CHUNK   # 32
    n_qc = s // QC         # 4

    # chunk-contiguous DRAM layouts for fast priority DMA
    qt_d = nc.dram_tensor("qt", [heads, n_qc, D, QC], F16,
                          kind="ExternalInput")
    kt_d = nc.dram_tensor("kt", [heads, 4, D, s // 4], F16,
                          kind="ExternalInput")
    v_d = nc.dram_tensor("v", [heads, 4, 128, n_kt // 4, D], F16,
                         kind="ExternalInput")
    out_d = nc.dram_tensor("out", [heads, D, s], F32, kind="ExternalOutput")
    den_d = nc.dram_tensor("den", [heads, s], F32, kind="ExternalOutput")

    with tile.TileContext(nc) as tc, ExitStack() as ctx:
        sb = ctx.enter_context(tc.tile_pool(name="sb", bufs=1))
        ps = ctx.enter_context(tc.tile_pool(name="ps", bufs=1, space="PSUM"))

        ones_h = sb.tile([128, 1], F16, tag="ones_h", bufs=1)
        nc.vector.memset(ones_h[:], 1.0)
        # warm the gpsimd ext-isa IRAM (~6us) during the DMA ramp
        warm = sb.tile([128, 64], F32, tag="warm", bufs=1)
        nc.vector.memset(warm[:], 0.0)
        nc.gpsimd.partition_all_reduce(warm[:], warm[:], 128,
                                       bass_isa.ReduceOp.add)

        def load_head(h, first=False):
            qt_sb = sb.tile([D, s], F16, tag="qt", name="qt_sb", bufs=2)
            kt_sb = sb.tile([D, s], F16, tag="kt", name="kt_sb", bufs=2)
            v_sb = sb.tile([128, n_kt, D], F16, tag="v", name="v_sb", bufs=2)
            ks = s // 4
            kg = n_kt // 4
            nc.sync.dma_start(out=kt_sb[:, 0:ks], in_=kt_d[h, 0])
            nc.sync.dma_start(out=qt_sb[:, 0:QC], in_=qt_d[h, 0])
            nc.sync.dma_start(out=v_sb[:, 0:kg, :], in_=v_d[h, 0])
            for c in range(1, 4):
                nc.sync.dma_start(out=kt_sb[:, c * ks:(c + 1) * ks],
                                  in_=kt_d[h, c])
                nc.sync.dma_start(out=v_sb[:, c * kg:(c + 1) * kg, :],
                                  in_=v_d[h, c])
            for c in range(1, n_qc):
                nc.sync.dma_start(out=qt_sb[:, c * QC:(c + 1) * QC],
                                  in_=qt_d[h, c])
            return qt_sb, kt_sb, v_sb

        heads_sb = [load_head(0, first=True)]

        # Deferred epilogue work, interleaved into the next chunk's stream
        # so psum drains and the reduction tail never stall PE or ACT.
        pending = []

        for h in range(heads):
            qt_sb, kt_sb, v_sb = heads_sb[h]
            if h + 1 < heads:
                heads_sb.append(load_head(h + 1))
            for qc in range(n_qc):
                q0 = qc * QC
                last_unit = (h == heads - 1) and (qc == n_qc - 1)
                out_ps = [ps.tile([D, NMM], F32, tag="outp",
                                  name=f"out_ps{j}", bufs=2)
                          for j in range(QC // NMM)]
                # three accumulation chains: DVE even/odd + GPSIMD
                accs = {"d0": None, "d1": None, "g": None}
                pv_queue = []
                chain_queue = []

                def issue_chain(kt, pt):
                    if kt in GPS_KTS:
                        chain, eng = "g", nc.gpsimd
                    else:
                        chain, eng = f"d{kt & 1}", nc.vector
                    if accs[chain] is None:
                        accs[chain] = pt
                    else:
                        a = sb.tile([128, QC], F16, tag="acc" + chain,
                                    name="acc" + chain, bufs=2)
                        eng.tensor_add(a[:], accs[chain][:], pt[:])
                        accs[chain] = a

                def issue_pv(kt, pt, out_ps=out_ps):
                    lhs_v = v_sb[:, kt, :]
                    for j in range(QC // NMM):
                        nc.tensor.matmul(
                            out_ps[j][:],
                            lhs_v,
                            pt[:, j * NMM:(j + 1) * NMM],
                            start=(kt == 0), stop=(kt == n_kt - 1))

                for kt in range(n_kt):
                    k0 = kt * KT_CHUNK
                    if kt in DVE_KTS:
                        st = ps.tile([128, QC], F32, tag="std", name="std",
                                     bufs=1)
                    else:
                        st = ps.tile([128, QC], F32, tag="st", name="st",
                                     bufs=2)
                    lhs_k = kt_sb[:, k0:k0 + KT_CHUNK]
                    for j in range(QC // NMM):
                        nc.tensor.matmul(
                            st[:, j * NMM:(j + 1) * NMM],
                            lhs_k,
                            qt_sb[:, q0 + j * NMM:q0 + (j + 1) * NMM],
                            start=True, stop=True)
                    pt = sb.tile([128, QC], F16, tag="pt", name="pt", bufs=14)
                    if kt in DVE_KTS:
                        # two-term Schraudolph: i2 = i1 + 512 exactly (the
                        # 0.5-binade phase shift is integer in i16 domain),
                        # so st (psum) is held by only one DVE pass
                        i1 = sb.tile([128, QC], I16, tag="i1", name="i1",
                                     bufs=2)
                        i2 = sb.tile([128, QC], I16, tag="i2", name="i2",
                                     bufs=2)
                        nc.vector.tensor_scalar(
                            i1[:], st[:], SCH_A, SCH_B1,
                            mybir.AluOpType.mult, mybir.AluOpType.add)
                        nc.vector.tensor_scalar_add(i2[:], i1[:], 512)
                        nc.vector.tensor_add(pt[:], i1[:].bitcast(F16),
                                             i2[:].bitcast(F16))
                    else:
                        nc.scalar.activation(
                            pt[:], st[:], mybir.ActivationFunctionType.Exp,
                            scale=SCALE)
                    chain_queue.append((kt, pt))
                    if kt >= CHAIN_LAG:
                        issue_chain(*chain_queue.pop(0))
                    pv_queue.append((kt, pt))
                    if kt >= PV_LAG:
                        issue_pv(*pv_queue.pop(0))
                        # final chunk: drain the PV backlog early so the
                        # tail after the last EXP is short (ACT does not
                        # depend on PV; only the last chunk pays PE lag)
                        if last_unit and kt >= 20 and pv_queue:
                            issue_pv(*pv_queue.pop(0))
                    if kt >= 2 and pending:
                        pending.pop(0)()
                while chain_queue:
                    issue_chain(*chain_queue.pop(0))
                while pv_queue:
                    issue_pv(*pv_queue.pop(0))

                last = last_unit

                def finish(out_ps=out_ps, accs=accs, h=h, q0=q0, last=last):
                    out_sb = sb.tile([D, QC], F32, tag="out_sb",
                                     name="out_sb", bufs=3)
                    accm = sb.tile([128, QC], F16, tag="accm",
                                   name="accm", bufs=2)
                    accm2 = sb.tile([128, QC], F16, tag="accm2",
                                    name="accm2", bufs=2)

                    def c_out0():
                        # on the last chunk ACT is idle; run the copies on
                        # both engines and DMA each half as it lands
                        if last:
                            nc.scalar.copy(out_sb[:, 0:NMM], out_ps[0][:])
                            nc.sync.dma_start(
                                out=out_d[h][:, q0:q0 + NMM],
                                in_=out_sb[:, 0:NMM])
                        else:
                            nc.vector.tensor_copy(out_sb[:, 0:NMM],
                                                  out_ps[0][:])

                    def c_out1():
                        nc.vector.tensor_copy(out_sb[:, NMM:QC], out_ps[1][:])
                        if last:
                            nc.sync.dma_start(
                                out=out_d[h][:, q0 + NMM:q0 + QC],
                                in_=out_sb[:, NMM:QC])
                        else:
                            nc.sync.dma_start(
                                out=out_d[h][:, q0:q0 + QC], in_=out_sb[:])

                    def c_fold():
                        nc.vector.tensor_add(accm[:], accs["d0"][:],
                                             accs["d1"][:])
                        if accs["g"] is not None:
                            nc.vector.tensor_add(accm2[:], accm[:],
                                                 accs["g"][:])

                    def acc_final():
                        return accm2 if accs["g"] is not None else accm

                    if last:
                        den_sb = sb.tile([1, QC], F32, tag="den_sb",
                                         name="den_sb", bufs=1)
                        dps = [ps.tile([1, NMM], F32, tag="outp",
                                       name=f"dps{j}", bufs=2)
                               for j in range(QC // NMM)]

                        def c_den():
                            am = acc_final()
                            for j in range(QC // NMM):
                                nc.tensor.matmul(
                                    dps[j][:],
                                    ones_h[:],
                                    am[:, j * NMM:(j + 1) * NMM],
                                    start=True, stop=True)
                                nc.scalar.copy(
                                    den_sb[:, j * NMM:(j + 1) * NMM],
                                    dps[j][:])
                            nc.sync.dma_start(
                                out=den_d[h:h + 1, q0:q0 + QC],
                                in_=den_sb[:])
                    else:
                        den_red = sb.tile([128, QC], F32, tag="den_red",
                                          name="den_red", bufs=2)

                        def c_den():
                            nc.gpsimd.partition_all_reduce(
                                den_red[:], acc_final()[:], 128,
                                bass_isa.ReduceOp.add)
                            nc.sync.dma_start(
                                out=den_d[h:h + 1, q0:q0 + QC],
                                in_=den_red[0:1, :])

                    if last:
                        return [c_out0, c_fold, c_out1, c_den]
                    return [c_out0, c_out1, c_fold, c_den]

                pending.extend(finish())
        while pending:
            pending.pop(0)()

    nc.compile()
    return nc


def _install_ntff_hook():
    """Provide antenv.axon_hooks (absent in this image) so that
    run_bass_kernel_spmd(trace=True) can capture NTFF profiles via the
    axon .so — mirrors trn_agent_boot.trn_boot._ntff_profile_via_ctypes."""
    try:
        from antenv.axon_hooks import get_axon_ntff_profile_hook  # noqa: F401
        return
    except ImportError:
        pass
    import contextlib
    import ctypes
    import types

    so_path = "/opt/axon/libaxon_pjrt.so"
    lib = ctypes.CDLL(so_path)
    if not hasattr(lib, "axon_start_nrt_profile"):
        return
    lib.axon_start_nrt_profile.argtypes = [
        ctypes.POINTER(ctypes.c_int64), ctypes.c_size_t]
    lib.axon_start_nrt_profile.restype = ctypes.c_int64
    lib.axon_stop_nrt_profile.argtypes = [ctypes.c_char_p]
    lib.axon_stop_nrt_profile.restype = ctypes.c_int64

    @contextlib.contextmanager
    def _hook(output_dir, device_ids):
        import jax
        jax.devices()
        if device_ids:
            ids = (ctypes.c_int64 * len(device_ids))(*device_ids)
            rc = lib.axon_start_nrt_profile(ids, len(device_ids))
        else:
            rc = lib.axon_start_nrt_profile(None, 0)
        if rc != 0:
            raise RuntimeError(f"axon_start_nrt_profile rc={rc}")
        try:
            yield
        finally:
            n = lib.axon_stop_nrt_profile(str(output_dir).encode())
            print(f"ntff profile: {n} file(s) written to {output_dir}")

    mod = types.ModuleType("antenv.axon_hooks")
    mod.get_axon_ntff_profile_hook = lambda: _hook
    mod.set_axon_ntff_profile_hook = lambda h: None
    import antenv
    sys.modules["antenv.axon_hooks"] = mod
    antenv.axon_hooks = mod


_CACHE = {}


def _get_program():
    key = "main"
    if key not in _CACHE:
        _CACHE[key] = build_program()
    return _CACHE[key]


def kernel(query, key, value, trace=False, **trace_kwargs):
    assert query.shape == (1, S, H, D)
    nc = _get_program()

    q = np.asarray(query, dtype=np.float32)[0]   # [S, H, D]
    k = np.asarray(key, dtype=np.float32)[0]
    v = np.asarray(value, dtype=np.float32)[0]

    n_kt = S // KT_CHUNK
    in_maps = []
    for c in range(N_CORES):
        hs = slice(c * HEADS_PER_CORE, (c + 1) * HEADS_PER_CORE)
        # [S, h, D] -> [h, D, S] -> chunked fp16
        qt = np.ascontiguousarray(
            q[:, hs, :].transpose(1, 2, 0)).astype(np.float16)
        kt = np.ascontiguousarray(
            k[:, hs, :].transpose(1, 2, 0)).astype(np.float16)
        qt = np.ascontiguousarray(
            qt.reshape(HEADS_PER_CORE, D, S // QC, QC).transpose(0, 2, 1, 3))
        kt = np.ascontiguousarray(
            kt.reshape(HEADS_PER_CORE, D, 4, S // 4).transpose(0, 2, 1, 3))
        # [S, h, D] -> [h, 4, 128, n_kt/4, D]: s = kt*128 + p
        vv = np.ascontiguousarray(
            v[:, hs, :].transpose(1, 0, 2)
            .reshape(HEADS_PER_CORE, 4, n_kt // 4, 128, D)
            .transpose(0, 1, 3, 2, 4)).astype(np.float16)
        in_maps.append({"qt": qt, "kt": kt, "v": vv})

    if trace:
        _install_ntff_hook()
    res = run_bass_kernel_spmd(nc, in_maps, core_ids=list(range(N_CORES)),
                               trace=trace, **trace_kwargs)

    out = np.empty((1, S, H, D), dtype=np.float32)
    for c in range(N_CORES):
        o = res.results[c]["out"]    # [h, D, S] unnormalized f32
        den = res.results[c]["den"]  # [h, S] f32
        for i in range(HEADS_PER_CORE):
            out[0, :, c * HEADS_PER_CORE + i, :] = (o[i] / den[i][None, :]).T
    if trace:
        kernel.last_results = res
    return out


# revision 20
# speedup vs baseline: 1.0030x; 1.0030x over previous
"""Trainium2 Bass kernel: full (non-causal) softmax attention.

Input:  query/key/value [1, 4096, 16, 128] f32 (B, S, H, D).
Output: [1, 4096, 16, 128] f32 = softmax(Q K^T / sqrt(D)) V per head.

Sharding: 16 heads over 8 cores -> 2 heads per core, no collectives.
Host pre-transposes Q,K per head to chunked [D, S] fp16; the device
returns the UN-normalized attention output [D, S] f32 plus the softmax
denominator row [S] f32; the host does the final divide.

Device pipeline, per head, per query-chunk QC (1024 queries), kt in 32
key-chunks (128 keys):
  ST[kt] = scores^T: psum f32 [128k, QC]   (two N=512 fp16 matmuls)
  PT[kt] = exp(ST/sqrt(128)) -> sbuf fp16:
     - most tiles on ACT (the 1.0us/tile throughput floor)
     - DVE_KTS tiles on DVE instead: two tensor_scalar exp2 bit-trick
       terms (i16 = st*a + b_i, bitcast fp16) summed with weights
       2^d1+2^d2 = 1/(1+mu) -- a mean-corrected two-phase Schraudolph
       with |rel err| <= 1.5% on those chunks only (~4e-3 end-to-end)
  OUT += V_kt^T @ PT[kt]   (fp16 matmuls, fp32 psum, issued PV_LAG late)
  den: PT accumulated in three fp16 chains (2x DVE, 1x GPSIMD), merged,
  then GPSIMD partition_all_reduce -> den row (no PSUM banks needed,
  freeing st to triple-buffer).
Epilogues are deferred into the next chunk's iterations; the last chunk
folds den via ones-matmuls into spare psum and drains on ACT (idle).
"""

import os
import sys
from contextlib import ExitStack

import numpy as np

sys.path.insert(0, "/opt/trn_rl_repo")

import concourse.bacc as bacc
import concourse.bass as bass
import concourse.bass_isa as bass_isa
import concourse.tile as tile
from concourse import mybir
from concourse.bass_utils import run_bass_kernel_spmd

N_CORES = 8
S = 4096
H = 16
D = 128
HEADS_PER_CORE = H // N_CORES  # 2
KT_CHUNK = 128                  # keys per score tile (psum partition dim)
QC = 1024                       # queries per super-chunk (ACT tile free dim)
NMM = 512                       # moving free dim per matmul (psum bank, f32)
SCALE = float(D) ** -0.5
LOG2E = 1.4426950408889634
PV_LAG = 6                      # PV matmuls trail QK by this many kt steps
CHAIN_LAG = 4                   # den-chain adds trail pt production, so the
                                # DVE-exp psum reads sit early in DVE's FIFO

# two-term Schraudolph constants (fp16 bit layout: bias 15, 10-bit mantissa)
# weights 2^D1 + 2^D2 = 1/(1+mu) cancel the mean of the mantissa-linear
# error; the half-period phase offset cancels its fundamental oscillation.
_SCH_MU = 0.04068
_SCH_D1 = -np.log2(1.0 + np.sqrt(2.0)) - np.log2(1.0 + _SCH_MU)
_SCH_D2 = _SCH_D1 + 0.5
SCH_A = float(SCALE * LOG2E * 1024.0)
SCH_B1 = float((15.0 + _SCH_D1) * 1024.0)
SCH_B2 = float((15.0 + _SCH_D2) * 1024.0)

DVE_KTS = (10, 21)              # kt tiles whose exp runs on DVE
GPS_KTS = ()                    # kt tiles whose den-chain add is GPSIMD
                                # (GPS adds contend with DVE's SBUF port)

F32 = mybir.dt.float32
F16 = mybir.dt.float16
I16 = mybir.dt.int16


def build_program(s=S, heads=HEADS_PER_CORE):
    nc = bacc.Bacc("TRN2", target_bir_lowering=False, debug=False,
                   num_devices=N_CORES)

    n_kt = s // KT_CHUNK   # 32
    n_qc = s // QC         # 4

    # chunk-contiguous DRAM layouts for fast priority DMA
    qt_d = nc.dram_tensor("qt", [heads, n_qc, D, QC], F16,
                          kind="ExternalInput")
    kt_d = nc.dram_tensor("kt", [heads, 4, D, s // 4], F16,
                          kind="ExternalInput")
    v_d = nc.dram_tensor("v", [heads, 4, 128, n_kt // 4, D], F16,
                         kind="ExternalInput")
    kt0_d = nc.dram_tensor("kt0", [D, KT_CHUNK], F16, kind="ExternalInput")
    qt0_d = nc.dram_tensor("qt0", [D, NMM], F16, kind="ExternalInput")
    out_d = nc.dram_tensor("out", [heads, D, s], F32, kind="ExternalOutput")
    den_d = nc.dram_tensor("den", [heads, s], F32, kind="ExternalOutput")

    with tile.TileContext(nc) as tc, ExitStack() as ctx:
        sb = ctx.enter_context(tc.tile_pool(name="sb", bufs=1))
        ps = ctx.enter_context(tc.tile_pool(name="ps", bufs=1, space="PSUM"))

        ones_h = sb.tile([128, 1], F16, tag="ones_h", bufs=1)
        nc.vector.memset(ones_h[:], 1.0)
        # warm the gpsimd ext-isa IRAM (~6us) during the DMA ramp
        warm = sb.tile([128, 64], F32, tag="warm", bufs=1)
        nc.vector.memset(warm[:], 0.0)
        nc.gpsimd.partition_all_reduce(warm[:], warm[:], 128,
                                       bass_isa.ReduceOp.add)

        def load_head(h, first=False):
            qt_sb = sb.tile([D, s], F16, tag="qt", name="qt_sb", bufs=2)
            kt_sb = sb.tile([D, s], F16, tag="kt", name="kt_sb", bufs=2)
            v_sb = sb.tile([128, n_kt, D], F16, tag="v", name="v_sb", bufs=2)
            ks = s // 4
            kg = n_kt // 4
            if first:
                # tiny contiguous copies of the first score tile's inputs
                # land long before the full chunks
                nc.sync.dma_start(out=kt_sb[:, 0:KT_CHUNK], in_=kt0_d[:, :])
                nc.sync.dma_start(out=qt_sb[:, 0:NMM], in_=qt0_d[:, :])
                nc.sync.dma_start(out=qt_sb[:, NMM:QC],
                                  in_=qt_d[h, 0][:, NMM:QC])
                nc.sync.dma_start(out=kt_sb[:, KT_CHUNK:ks],
                                  in_=kt_d[h, 0][:, KT_CHUNK:ks])
            else:
                nc.sync.dma_start(out=kt_sb[:, 0:ks], in_=kt_d[h, 0])
                nc.sync.dma_start(out=qt_sb[:, 0:QC], in_=qt_d[h, 0])
            nc.sync.dma_start(out=v_sb[:, 0:kg, :], in_=v_d[h, 0])
            for c in range(1, 4):
                nc.sync.dma_start(out=kt_sb[:, c * ks:(c + 1) * ks],
                                  in_=kt_d[h, c])
                nc.sync.dma_start(out=v_sb[:, c * kg:(c + 1) * kg, :],
                                  in_=v_d[h, c])
            for c in range(1, n_qc):
                nc.sync.dma_start(out=qt_sb[:, c * QC:(c + 1) * QC],
                                  in_=qt_d[h, c])
            return qt_sb, kt_sb, v_sb

        heads_sb = [load_head(0, first=True)]

        # Deferred epilogue work, interleaved into the next chunk's stream
        # so psum drains and the reduction tail never stall PE or ACT.
        pending = []

        for h in range(heads):
            qt_sb, kt_sb, v_sb = heads_sb[h]
            if h + 1 < heads:
                heads_sb.append(load_head(h + 1))
            for qc in range(n_qc):
                q0 = qc * QC
                last_unit = (h == heads - 1) and (qc == n_qc - 1)
                out_ps = [ps.tile([D, NMM], F32, tag="outp",
                                  name=f"out_ps{j}", bufs=2)
                          for j in range(QC // NMM)]
                # three accumulation chains: DVE even/odd + GPSIMD
                accs = {"d0": None, "d1": None, "g": None}
                pv_queue = []
                chain_queue = []

                def issue_chain(kt, pt):
                    if kt in GPS_KTS:
                        chain, eng = "g", nc.gpsimd
                    else:
                        chain, eng = f"d{kt & 1}", nc.vector
                    if accs[chain] is None:
                        accs[chain] = pt
                    else:
                        a = sb.tile([128, QC], F16, tag="acc" + chain,
                                    name="acc" + chain, bufs=2)
                        eng.tensor_add(a[:], accs[chain][:], pt[:])
                        accs[chain] = a

                def issue_pv(kt, pt, out_ps=out_ps):
                    lhs_v = v_sb[:, kt, :]
                    for j in range(QC // NMM):
                        nc.tensor.matmul(
                            out_ps[j][:],
                            lhs_v,
                            pt[:, j * NMM:(j + 1) * NMM],
                            start=(kt == 0), stop=(kt == n_kt - 1))

                for kt in range(n_kt):
                    k0 = kt * KT_CHUNK
                    if kt in DVE_KTS:
                        st = ps.tile([128, QC], F32, tag="std", name="std",
                                     bufs=1)
                    else:
                        st = ps.tile([128, QC], F32, tag="st", name="st",
                                     bufs=2)
                    lhs_k = kt_sb[:, k0:k0 + KT_CHUNK]
                    for j in range(QC // NMM):
                        nc.tensor.matmul(
                            st[:, j * NMM:(j + 1) * NMM],
                            lhs_k,
                            qt_sb[:, q0 + j * NMM:q0 + (j + 1) * NMM],
                            start=True, stop=True)
                    pt = sb.tile([128, QC], F16, tag="pt", name="pt", bufs=14)
                    if kt in DVE_KTS:
                        # two-term Schraudolph: i2 = i1 + 512 exactly (the
                        # 0.5-binade phase shift is integer in i16 domain),
                        # so st (psum) is held by only one DVE pass
                        i1 = sb.tile([128, QC], I16, tag="i1", name="i1",
                                     bufs=2)
                        i2 = sb.tile([128, QC], I16, tag="i2", name="i2",
                                     bufs=2)
                        nc.vector.tensor_scalar(
                            i1[:], st[:], SCH_A, SCH_B1,
                            mybir.AluOpType.mult, mybir.AluOpType.add)
                        nc.vector.tensor_scalar_add(i2[:], i1[:], 512)
                        nc.vector.tensor_add(pt[:], i1[:].bitcast(F16),
                                             i2[:].bitcast(F16))
                    else:
                        nc.scalar.activation(
                            pt[:], st[:], mybir.ActivationFunctionType.Exp,
                            scale=SCALE)
                    chain_queue.append((kt, pt))
                    if kt >= CHAIN_LAG:
                        issue_chain(*chain_queue.pop(0))
                    pv_queue.append((kt, pt))
                    if kt >= PV_LAG:
                        issue_pv(*pv_queue.pop(0))
                    if kt >= 2 and pending:
                        pending.pop(0)()
                while chain_queue:
                    issue_chain(*chain_queue.pop(0))
                while pv_queue:
                    issue_pv(*pv_queue.pop(0))

                last = last_unit

                def finish(out_ps=out_ps, accs=accs, h=h, q0=q0, last=last):
                    out_sb = sb.tile([D, QC], F32, tag="out_sb",
                                     name="out_sb", bufs=3)
                    accm = sb.tile([128, QC], F16, tag="accm",
                                   name="accm", bufs=2)
                    accm2 = sb.tile([128, QC], F16, tag="accm2",
                                    name="accm2", bufs=2)

                    def c_out0():
                        # on the last chunk ACT is idle; run the copies on
                        # both engines and DMA each half as it lands
                        if last:
                            nc.scalar.copy(out_sb[:, 0:NMM], out_ps[0][:])
                            nc.sync.dma_start(
                                out=out_d[h][:, q0:q0 + NMM],
                                in_=out_sb[:, 0:NMM])
                        else:
                            nc.vector.tensor_copy(out_sb[:, 0:NMM],
                                                  out_ps[0][:])

                    def c_out1():
                        nc.vector.tensor_copy(out_sb[:, NMM:QC], out_ps[1][:])
                        if last:
                            nc.sync.dma_start(
                                out=out_d[h][:, q0 + NMM:q0 + QC],
                                in_=out_sb[:, NMM:QC])
                        else:
                            nc.sync.dma_start(
                                out=out_d[h][:, q0:q0 + QC], in_=out_sb[:])

                    def c_fold():
                        nc.vector.tensor_add(accm[:], accs["d0"][:],
                                             accs["d1"][:])
                        if accs["g"] is not None:
                            nc.vector.tensor_add(accm2[:], accm[:],
                                                 accs["g"][:])

                    def acc_final():
                        return accm2 if accs["g"] is not None else accm

                    if last:
                        den_sb = sb.tile([1, QC], F32, tag="den_sb",
                                         name="den_sb", bufs=1)
                        dps = [ps.tile([1, NMM], F32, tag="outp",
                                       name=f"dps{j}", bufs=2)
                               for j in range(QC // NMM)]

                        def c_den():
                            am = acc_final()
                            for j in range(QC // NMM):
                                nc.tensor.matmul(
                                    dps[j][:],
                                    ones_h[:],
                                    am[:, j * NMM:(j + 1) * NMM],
                                    start=True, stop=True)
                                nc.scalar.copy(
                                    den_sb[:, j * NMM:(j + 1) * NMM],
                                    dps[j][:])
                            nc.sync.dma_start(
                                out=den_d[h:h + 1, q0:q0 + QC],
                                in_=den_sb[:])
                    else:
                        den_red = sb.tile([128, QC], F32, tag="den_red",
                                          name="den_red", bufs=2)

                        def c_den():
                            nc.gpsimd.partition_all_reduce(
                                den_red[:], acc_final()[:], 128,
                                bass_isa.ReduceOp.add)
                            nc.sync.dma_start(
                                out=den_d[h:h + 1, q0:q0 + QC],
                                in_=den_red[0:1, :])

                    if last:
                        return [c_out0, c_fold, c_out1, c_den]
                    return [c_out0, c_out1, c_fold, c_den]

                pending.extend(finish())
        while pending:
            pending.pop(0)()

    nc.compile()
    return nc


def _install_ntff_hook():
    """Provide antenv.axon_hooks (absent in this image) so that
    run_bass_kernel_spmd(trace=True) can capture NTFF profiles via the
    axon .so — mirrors trn_agent_boot.trn_boot._ntff_profile_via_ctypes."""
    try:
        from antenv.axon_hooks import get_axon_ntff_profile_hook  # noqa: F401
        return
    except ImportError:
        pass
    import contextlib
    import ctypes
    import types

    so_path = "/opt/axon/libaxon_pjrt.so"
    lib = ctypes.CDLL(so_path)
    if not hasattr(lib, "axon_start_nrt_profile"):
        return
    lib.axon_start_nrt_profile.argtypes = [
        ctypes.POINTER(ctypes.c_int64), ctypes.c_size_t]
    lib.axon_start_nrt_profile.restype = ctypes.c_int64
    lib.axon_stop_nrt_profile.argtypes = [ctypes.c_char_p]
    lib.axon_stop_nrt_profile.restype = ctypes.c_int64

    @contextlib.contextmanager
    def _hook(output_dir, device_ids):
        import jax
        jax.devices()
        if device_ids:
            ids = (ctypes.c_int64 * len(device_ids))(*device_ids)
            rc = lib.axon_start_nrt_profile(ids, len(device_ids))
        else:
            rc = lib.axon_start_nrt_profile(None, 0)
        if rc != 0:
            raise RuntimeError(f"axon_start_nrt_profile rc={rc}")
        try:
            yield
        finally:
            n = lib.axon_stop_nrt_profile(str(output_dir).encode())
            print(f"ntff profile: {n} file(s) written to {output_dir}")

    mod = types.ModuleType("antenv.axon_hooks")
    mod.get_axon_ntff_profile_hook = lambda: _hook
    mod.set_axon_ntff_profile_hook = lambda h: None
    import antenv
    sys.modules["antenv.axon_hooks"] = mod
    antenv.axon_hooks = mod


_CACHE = {}


def _get_program():
    key = "main"
    if key not in _CACHE:
        _CACHE[key] = build_program()
    return _CACHE[key]


def kernel(query, key, value, trace=False, **trace_kwargs):
    assert query.shape == (1, S, H, D)
    nc = _get_program()

    q = np.asarray(query, dtype=np.float32)[0]   # [S, H, D]
    k = np.asarray(key, dtype=np.float32)[0]
    v = np.asarray(value, dtype=np.float32)[0]

    n_kt = S // KT_CHUNK
    in_maps = []
    for c in range(N_CORES):
        hs = slice(c * HEADS_PER_CORE, (c + 1) * HEADS_PER_CORE)
        # [S, h, D] -> [h, D, S] -> chunked fp16
        qt = np.ascontiguousarray(
            q[:, hs, :].transpose(1, 2, 0)).astype(np.float16)
        kt = np.ascontiguousarray(
            k[:, hs, :].transpose(1, 2, 0)).astype(np.float16)
        qt = np.ascontiguousarray(
            qt.reshape(HEADS_PER_CORE, D, S // QC, QC).transpose(0, 2, 1, 3))
        kt = np.ascontiguousarray(
            kt.reshape(HEADS_PER_CORE, D, 4, S // 4).transpose(0, 2, 1, 3))
        # [S, h, D] -> [h, 4, 128, n_kt/4, D]: s = kt*128 + p
        vv = np.ascontiguousarray(
            v[:, hs, :].transpose(1, 0, 2)
            .reshape(HEADS_PER_CORE, 4, n_kt // 4, 128, D)
            .transpose(0, 1, 3, 2, 4)).astype(np.float16)
        in_maps.append({"qt": qt, "kt": kt, "v": vv,
                        "kt0": np.ascontiguousarray(kt[0, 0][:, 0:KT_CHUNK]),
                        "qt0": np.ascontiguousarray(qt[0, 0][:, 0:NMM])})

    if trace:
        _install_ntff_hook()
    res = run_bass_kernel_spmd(nc, in_maps, core_ids=list(range(N_CORES)),
                               trace=trace, **trace_kwargs)

    out = np.empty((1, S, H, D), dtype=np.float32)
    for c in range(N_CORES):
        o = res.results[c]["out"]    # [h, D, S] unnormalized f32
        den = res.results[c]["den"]  # [h, S] f32
        for i in range(HEADS_PER_CORE):
            out[0, :, c * HEADS_PER_CORE + i, :] = (o[i] / den[i][None, :]).T
    if trace:
        kernel.last_results = res
    return out


# revision 21
# speedup vs baseline: 1.0087x; 1.0057x over previous
"""Trainium2 Bass kernel: full (non-causal) softmax attention.

Input:  query/key/value [1, 4096, 16, 128] f32 (B, S, H, D).
Output: [1, 4096, 16, 128] f32 = softmax(Q K^T / sqrt(D)) V per head.

Sharding: 16 heads over 8 cores -> 2 heads per core, no collectives.
Host pre-transposes Q,K per head to chunked [D, S] fp16; the device
returns the UN-normalized attention output [D, S] f32 plus the softmax
denominator row [S] f32; the host does the final divide.

Device pipeline, per head, per query-chunk QC (1024 queries), kt in 32
key-chunks (128 keys):
  ST[kt] = scores^T: psum f32 [128k, QC]   (two N=512 fp16 matmuls)
  PT[kt] = exp(ST/sqrt(128)) -> sbuf fp16:
     - most tiles on ACT (the 1.0us/tile throughput floor)
     - DVE_KTS tiles on DVE instead: two tensor_scalar exp2 bit-trick
       terms (i16 = st*a + b_i, bitcast fp16) summed with weights
       2^d1+2^d2 = 1/(1+mu) -- a mean-corrected two-phase Schraudolph
       with |rel err| <= 1.5% on those chunks only (~4e-3 end-to-end)
  OUT += V_kt^T @ PT[kt]   (fp16 matmuls, fp32 psum, issued PV_LAG late)
  den: PT accumulated in two fp16 DVE chains, merged,
  then GPSIMD partition_all_reduce -> den row (no PSUM banks needed,
  freeing st to triple-buffer).
Epilogues are deferred into the next chunk's iterations; the last chunk
folds den via ones-matmuls into spare psum and drains on ACT (idle).
"""

import os
import sys
from contextlib import ExitStack

import numpy as np

sys.path.insert(0, "/opt/trn_rl_repo")

import concourse.bacc as bacc
import concourse.bass as bass
import concourse.bass_isa as bass_isa
import concourse.tile as tile
from concourse import mybir
from concourse.bass_utils import run_bass_kernel_spmd

N_CORES = 8
S = 4096
H = 16
D = 128
HEADS_PER_CORE = H // N_CORES  # 2
KT_CHUNK = 128                  # keys per score tile (psum partition dim)
QC = 1024                       # queries per super-chunk (ACT tile free dim)
NMM = 512                       # moving free dim per matmul (psum bank, f32)
SCALE = float(D) ** -0.5
LOG2E = 1.4426950408889634
PV_LAG = 6                      # PV matmuls trail QK by this many kt steps
CHAIN_LAG = 4                   # den-chain adds trail pt production, so the
                                # DVE-exp psum reads sit early in DVE's FIFO

# two-term Schraudolph constants (fp16 bit layout: bias 15, 10-bit mantissa)
# weights 2^D1 + 2^D2 = 1/(1+mu) cancel the mean of the mantissa-linear
# error; the half-period phase offset cancels its fundamental oscillation.
_SCH_MU = 0.04068
_SCH_D1 = -np.log2(1.0 + np.sqrt(2.0)) - np.log2(1.0 + _SCH_MU)
_SCH_D2 = _SCH_D1 + 0.5
SCH_A = float(SCALE * LOG2E * 1024.0)
SCH_B1 = float((15.0 + _SCH_D1) * 1024.0)
SCH_B2 = float((15.0 + _SCH_D2) * 1024.0)

DVE_KTS = (10, 21)              # kt tiles whose exp runs on DVE
GPS_KTS = ()                    # kt tiles whose den-chain add is GPSIMD
                                # (GPS adds contend with DVE's SBUF port)

F32 = mybir.dt.float32
F16 = mybir.dt.float16
I16 = mybir.dt.int16


def build_program(s=S, heads=HEADS_PER_CORE):
    nc = bacc.Bacc("TRN2", target_bir_lowering=False, debug=False,
                   num_devices=N_CORES)

    n_kt = s // KT_CHUNK   # 32
    n_qc = s // QC         # 4

    # chunk-contiguous DRAM layouts for fast priority DMA
    qt_d = nc.dram_tensor("qt", [heads, n_qc, D, QC], F16,
                          kind="ExternalInput")
    kt_d = nc.dram_tensor("kt", [heads, 4, D, s // 4], F16,
                          kind="ExternalInput")
    v_d = nc.dram_tensor("v", [heads, 4, 128, n_kt // 4, D], F16,
                         kind="ExternalInput")
    out_d = nc.dram_tensor("out", [heads, D, s], F32, kind="ExternalOutput")
    den_d = nc.dram_tensor("den", [heads, s], F32, kind="ExternalOutput")

    with tile.TileContext(nc) as tc, ExitStack() as ctx:
        sb = ctx.enter_context(tc.tile_pool(name="sb", bufs=1))
        ps = ctx.enter_context(tc.tile_pool(name="ps", bufs=1, space="PSUM"))

        ones_h = sb.tile([128, 1], F16, tag="ones_h", bufs=1)
        nc.vector.memset(ones_h[:], 1.0)
        # warm the gpsimd ext-isa IRAM (~6us) during the DMA ramp
        warm = sb.tile([128, 64], F32, tag="warm", bufs=1)
        nc.vector.memset(warm[:], 0.0)
        nc.gpsimd.partition_all_reduce(warm[:], warm[:], 128,
                                       bass_isa.ReduceOp.add)

        def load_head(h, first=False):
            qt_sb = sb.tile([D, s], F16, tag="qt", name="qt_sb", bufs=2)
            kt_sb = sb.tile([D, s], F16, tag="kt", name="kt_sb", bufs=2)
            v_sb = sb.tile([128, n_kt, D], F16, tag="v", name="v_sb", bufs=2)
            ks = s // 4
            kg = n_kt // 4
            nc.sync.dma_start(out=kt_sb[:, 0:ks], in_=kt_d[h, 0])
            nc.sync.dma_start(out=qt_sb[:, 0:QC], in_=qt_d[h, 0])
            nc.sync.dma_start(out=v_sb[:, 0:kg, :], in_=v_d[h, 0])
            for c in range(1, 4):
                nc.sync.dma_start(out=kt_sb[:, c * ks:(c + 1) * ks],
                                  in_=kt_d[h, c])
                nc.sync.dma_start(out=v_sb[:, c * kg:(c + 1) * kg, :],
                                  in_=v_d[h, c])
            for c in range(1, n_qc):
                nc.sync.dma_start(out=qt_sb[:, c * QC:(c + 1) * QC],
                                  in_=qt_d[h, c])
            return qt_sb, kt_sb, v_sb

        heads_sb = [load_head(0, first=True)]

        # Deferred epilogue work, interleaved into the next chunk's stream
        # so psum drains and the reduction tail never stall PE or ACT.
        pending = []

        for h in range(heads):
            qt_sb, kt_sb, v_sb = heads_sb[h]
            if h + 1 < heads:
                heads_sb.append(load_head(h + 1))
            for qc in range(n_qc):
                q0 = qc * QC
                last_unit = (h == heads - 1) and (qc == n_qc - 1)
                out_ps = [ps.tile([D, NMM], F32, tag="outp",
                                  name=f"out_ps{j}", bufs=2)
                          for j in range(QC // NMM)]
                # three accumulation chains: DVE even/odd + GPSIMD
                accs = {"d0": None, "d1": None, "g": None}
                pv_queue = []
                chain_queue = []

                def issue_chain(kt, pt):
                    if kt in GPS_KTS:
                        chain, eng = "g", nc.gpsimd
                    else:
                        chain, eng = f"d{kt & 1}", nc.vector
                    if accs[chain] is None:
                        accs[chain] = pt
                    else:
                        a = sb.tile([128, QC], F16, tag="acc" + chain,
                                    name="acc" + chain, bufs=2)
                        eng.tensor_add(a[:], accs[chain][:], pt[:])
                        accs[chain] = a

                def issue_pv(kt, pt, out_ps=out_ps):
                    lhs_v = v_sb[:, kt, :]
                    for j in range(QC // NMM):
                        nc.tensor.matmul(
                            out_ps[j][:],
                            lhs_v,
                            pt[:, j * NMM:(j + 1) * NMM],
                            start=(kt == 0), stop=(kt == n_kt - 1))

                for kt in range(n_kt):
                    k0 = kt * KT_CHUNK
                    if kt in DVE_KTS:
                        st = ps.tile([128, QC], F32, tag="std", name="std",
                                     bufs=1)
                    else:
                        st = ps.tile([128, QC], F32, tag="st", name="st",
                                     bufs=2)
                    lhs_k = kt_sb[:, k0:k0 + KT_CHUNK]
                    for j in range(QC // NMM):
                        nc.tensor.matmul(
                            st[:, j * NMM:(j + 1) * NMM],
                            lhs_k,
                            qt_sb[:, q0 + j * NMM:q0 + (j + 1) * NMM],
                            start=True, stop=True)
                    pt = sb.tile([128, QC], F16, tag="pt", name="pt", bufs=14)
                    if kt in DVE_KTS:
                        # two-term Schraudolph: i2 = i1 + 512 exactly (the
                        # 0.5-binade phase shift is integer in i16 domain),
                        # so st (psum) is held by only one DVE pass
                        i1 = sb.tile([128, QC], I16, tag="i1", name="i1",
                                     bufs=2)
                        i2 = sb.tile([128, QC], I16, tag="i2", name="i2",
                                     bufs=2)
                        nc.vector.tensor_scalar(
                            i1[:], st[:], SCH_A, SCH_B1,
                            mybir.AluOpType.mult, mybir.AluOpType.add)
                        nc.vector.tensor_scalar_add(i2[:], i1[:], 512)
                        nc.vector.tensor_add(pt[:], i1[:].bitcast(F16),
                                             i2[:].bitcast(F16))
                    else:
                        nc.scalar.activation(
                            pt[:], st[:], mybir.ActivationFunctionType.Exp,
                            scale=SCALE)
                    chain_queue.append((kt, pt))
                    if kt >= CHAIN_LAG:
                        issue_chain(*chain_queue.pop(0))
                    pv_queue.append((kt, pt))
                    if kt >= PV_LAG:
                        issue_pv(*pv_queue.pop(0))
                    if kt >= 2 and pending:
                        pending.pop(0)()
                while chain_queue:
                    issue_chain(*chain_queue.pop(0))
                while pv_queue:
                    issue_pv(*pv_queue.pop(0))

                last = last_unit

                def finish(out_ps=out_ps, accs=accs, h=h, q0=q0, last=last):
                    out_sb = sb.tile([D, QC], F32, tag="out_sb",
                                     name="out_sb", bufs=3)
                    accm = sb.tile([128, QC], F16, tag="accm",
                                   name="accm", bufs=2)
                    accm2 = sb.tile([128, QC], F16, tag="accm2",
                                    name="accm2", bufs=2)

                    def c_out0():
                        # on the last chunk ACT is idle; run the copies on
                        # both engines and DMA each half as it lands
                        if last:
                            nc.scalar.copy(out_sb[:, 0:NMM], out_ps[0][:])
                            nc.sync.dma_start(
                                out=out_d[h][:, q0:q0 + NMM],
                                in_=out_sb[:, 0:NMM])
                        else:
                            nc.vector.tensor_copy(out_sb[:, 0:NMM],
                                                  out_ps[0][:])

                    def c_out1():
                        nc.vector.tensor_copy(out_sb[:, NMM:QC], out_ps[1][:])
                        if last:
                            nc.sync.dma_start(
                                out=out_d[h][:, q0 + NMM:q0 + QC],
                                in_=out_sb[:, NMM:QC])
                        else:
                            nc.sync.dma_start(
                                out=out_d[h][:, q0:q0 + QC], in_=out_sb[:])

                    def c_fold():
                        nc.vector.tensor_add(accm[:], accs["d0"][:],
                                             accs["d1"][:])
                        if accs["g"] is not None:
                            nc.vector.tensor_add(accm2[:], accm[:],
                                                 accs["g"][:])

                    def acc_final():
                        return accm2 if accs["g"] is not None else accm

                    if last:
                        den_sb = sb.tile([1, QC], F32, tag="den_sb",
                                         name="den_sb", bufs=1)
                        dps = [ps.tile([1, NMM], F32, tag="outp",
                                       name=f"dps{j}", bufs=2)
                               for j in range(QC // NMM)]

                        def c_den():
                            am = acc_final()
                            for j in range(QC // NMM):
                                nc.tensor.matmul(
                                    dps[j][:],
                                    ones_h[:],
                                    am[:, j * NMM:(j + 1) * NMM],
                                    start=True, stop=True)
                                nc.scalar.copy(
                                    den_sb[:, j * NMM:(j + 1) * NMM],
                                    dps[j][:])
                            nc.sync.dma_start(
                                out=den_d[h:h + 1, q0:q0 + QC],
                                in_=den_sb[:])
                    else:
                        den_red = sb.tile([128, QC], F32, tag="den_red",
                                          name="den_red", bufs=2)

                        def c_den():
                            nc.gpsimd.partition_all_reduce(
                                den_red[:], acc_final()[:], 128,
                                bass_isa.ReduceOp.add)
                            nc.sync.dma_start(
                                out=den_d[h:h + 1, q0:q0 + QC],
                                in_=den_red[0:1, :])

                    if last:
                        return [c_out0, c_fold, c_out1, c_den]
                    return [c_out0, c_out1, c_fold, c_den]

                pending.extend(finish())
        while pending:
            pending.pop(0)()

    nc.compile()
    return nc


def _install_ntff_hook():
    """Provide antenv.axon_hooks (absent in this image) so that
    run_bass_kernel_spmd(trace=True) can capture NTFF profiles via the
    axon .so — mirrors trn_agent_boot.trn_boot._ntff_profile_via_ctypes."""
    try:
        from antenv.axon_hooks import get_axon_ntff_profile_hook  # noqa: F401
        return
    except ImportError:
        pass
    import contextlib
    import ctypes
    import types

    so_path = "/opt/axon/libaxon_pjrt.so"
    lib = ctypes.CDLL(so_path)
    if not hasattr(lib, "axon_start_nrt_profile"):
        return
    lib.axon_start_nrt_profile.argtypes = [
        ctypes.POINTER(ctypes.c_int64), ctypes.c_size_t]
    lib.axon_start_nrt_profile.restype = ctypes.c_int64
    lib.axon_stop_nrt_profile.argtypes = [ctypes.c_char_p]
    lib.axon_stop_nrt_profile.restype = ctypes.c_int64

    @contextlib.contextmanager
    def _hook(output_dir, device_ids):
        import jax
        jax.devices()
        if device_ids:
            ids = (ctypes.c_int64 * len(device_ids))(*device_ids)
            rc = lib.axon_start_nrt_profile(ids, len(device_ids))
        else:
            rc = lib.axon_start_nrt_profile(None, 0)
        if rc != 0:
            raise RuntimeError(f"axon_start_nrt_profile rc={rc}")
        try:
            yield
        finally:
            n = lib.axon_stop_nrt_profile(str(output_dir).encode())
            print(f"ntff profile: {n} file(s) written to {output_dir}")

    mod = types.ModuleType("antenv.axon_hooks")
    mod.get_axon_ntff_profile_hook = lambda: _hook
    mod.set_axon_ntff_profile_hook = lambda h: None
    import antenv
    sys.modules["antenv.axon_hooks"] = mod
    antenv.axon_hooks = mod


_CACHE = {}


def _get_program():
    key = "main"
    if key not in _CACHE:
        _CACHE[key] = build_program()
    return _CACHE[key]


def kernel(query, key, value, trace=False, **trace_kwargs):
    assert query.shape == (1, S, H, D)
    nc = _get_program()

    q = np.asarray(query, dtype=np.float32)[0]   # [S, H, D]
    k = np.asarray(key, dtype=np.float32)[0]
    v = np.asarray(value, dtype=np.float32)[0]

    n_kt = S // KT_CHUNK
    in_maps = []
    for c in range(N_CORES):
        hs = slice(c * HEADS_PER_CORE, (c + 1) * HEADS_PER_CORE)
        # [S, h, D] -> [h, D, S] -> chunked fp16
        qt = np.ascontiguousarray(
            q[:, hs, :].transpose(1, 2, 0)).astype(np.float16)
        kt = np.ascontiguousarray(
            k[:, hs, :].transpose(1, 2, 0)).astype(np.float16)
        qt = np.ascontiguousarray(
            qt.reshape(HEADS_PER_CORE, D, S // QC, QC).transpose(0, 2, 1, 3))
        kt = np.ascontiguousarray(
            kt.reshape(HEADS_PER_CORE, D, 4, S // 4).transpose(0, 2, 1, 3))
        # [S, h, D] -> [h, 4, 128, n_kt/4, D]: s = kt*128 + p
        vv = np.ascontiguousarray(
            v[:, hs, :].transpose(1, 0, 2)
            .reshape(HEADS_PER_CORE, 4, n_kt // 4, 128, D)
            .transpose(0, 1, 3, 2, 4)).astype(np.float16)
        in_maps.append({"qt": qt, "kt": kt, "v": vv})

    if trace:
        _install_ntff_hook()
    res = run_bass_kernel_spmd(nc, in_maps, core_ids=list(range(N_CORES)),
                               trace=trace, **trace_kwargs)

    out = np.empty((1, S, H, D), dtype=np.float32)
    for c in range(N_CORES):
        o = res.results[c]["out"]    # [h, D, S] unnormalized f32
        den = res.results[c]["den"]  # [h, S] f32
        for i in range(HEADS_PER_CORE):
            out[0, :, c * HEADS_PER_CORE + i, :] = (o[i] / den[i][None, :]).T
    if trace:
        kernel.last_results = res
    return out
